# revision 38
# baseline (speedup 1.0000x reference)
"""Multi-head attention (B=16, N=1024, D=768, H=12) on 8 TRN2 NeuronCores.

Strategy: data-parallel over batch (2 batches per core, no collectives).
Per-core kernel, all matmuls on TensorE in bf16 (PE cost on TRN2 is
out-free-size cycles regardless of K/M, so every GEMM is tiled to its
F-cycle minimum):
  - QKV projection from pre-transposed x (feature-major xT in SBUF, bf16).
    Startup DMAs are split/interleaved (x quarters, V/Q/K weight sections)
    so the first V-projection matmul starts ~3us in.
  - Scores computed directly TRANSPOSED (S^T[k, q]) so the exp output
    P^T lands in exactly the layout the PV matmul needs as rhs.
  - exp on ScalarE with the 1/sqrt(hd) scale folded in (no max-subtract:
    scores are O(5) for this input distribution, far from fp32 overflow).
  - Softmax denominators WITHOUT PE work: P^T is folded 8->1 over the
    k-blocks by a 3-level bf16 add-tree on VectorE (runs at the 2x DVE
    rate), then a single partition_all_reduce on the idle GpSimd/Pool
    engine sums over partitions and broadcasts the result everywhere;
    the 1/denominator normalization is fused into the PV PSUM->SBUF
    copyback on VectorE.  (The previous ones-matmul denominator cost
    2x F=512 PE matmuls per k-block - 82us/core of pure PE overhead.)
  - PV col-tiled (two heads per PSUM bank, M=64 each) in bf16 producing
    O^T feature-major, which feeds the output projection (bf16) without
    any transposes.
  - Software pipelining: chunk c's PV/fold/finalize run interleaved into
    chunk c+1's score emission, carried ACROSS batch boundaries; the next
    batch's x DMA + V projection and the previous batch's output
    projection are interleaved into per-k-block slots, so the PE never
    drains between batches (95.9% PE occupancy, CoreSim).
"""

import sys

sys.path.insert(0, "/opt/trn_rl_repo")

import numpy as np
import ml_dtypes

import concourse.mybir as mybir
import concourse.tile as tile
from concourse import bacc
from concourse import bass_isa
from concourse.bass_utils import run_bass_kernel_spmd

F32 = mybir.dt.float32
F32R = mybir.dt.float32r
BF16 = mybir.dt.bfloat16

B, N, D = 16, 1024, 768
H = 12
HD = D // H          # 64
SCALE = float(HD) ** -0.5   # 0.125
NCORES = 8
BL = B // NCORES     # batches per core
ROWS = BL * N        # 2048 rows per core
DT = D // 128        # 6 d-tiles
NP = H // 2          # 6 head pairs
EXP = mybir.ActivationFunctionType.Exp
MUL = mybir.AluOpType.mult
ADD = mybir.AluOpType.add


def build_nc(repeat=1, qk_bf16=True, probe=None):
    nc = bacc.Bacc("TRN2", target_bir_lowering=False, debug=False)

    QKDT = BF16 if qk_bf16 else F32R
    xT_ext = nc.declare_dram_parameter("xT", [D, ROWS], BF16 if qk_bf16 else F32, isOutput=False)
    wqkvT_ext = nc.declare_dram_parameter("wqkvT", [D, 3 * D], BF16 if qk_bf16 else F32, isOutput=False)
    wprojT_ext = nc.declare_dram_parameter("wprojT", [D, D], BF16, isOutput=False)
    bias_ext = nc.declare_dram_parameter("biasb", [128, D], F32, isOutput=False)
    out_ext = nc.declare_dram_parameter("out", [ROWS, D], F32, isOutput=True)

    NB = repeat * BL

    with tile.TileContext(nc) as tc:
        with (
            tc.tile_pool(name="const", bufs=1) as constp,
            tc.tile_pool(name="work", bufs=1) as work,
            tc.tile_pool(name="mmps", bufs=2, space="PSUM") as mmps,
            tc.tile_pool(name="stps", bufs=2, space="PSUM") as stps,
            tc.tile_pool(name="pvps", bufs=2, space="PSUM") as pvps,
        ):
            # ---- constants / startup DMAs ----
            # Ordered so the PE can start as early as possible: first 512 rows
            # of x, then the V-section of wqkv (the first compute emitted is
            # the V projection), then the rest.
            def load_xT_part(xT_sb, b, h0, hl):
                xs = xT_ext[:, b * N + h0:b * N + h0 + hl].rearrange(
                    "(o p) r -> p o r", p=128)
                nc.sync.dma_start(
                    xT_sb[:, :, h0:h0 + hl], xs if qk_bf16 else xs.bitcast(F32R))

            wqkvT_sb = constp.tile([128, DT, 3 * D], QKDT)
            wq_src = wqkvT_ext.rearrange("(o p) e -> p o e", p=128)

            def load_wqkv_part(e0, ew):
                src = wq_src[:, :, e0:e0 + ew]
                nc.sync.dma_start(
                    wqkvT_sb[:, :, e0:e0 + ew], src if qk_bf16 else src.bitcast(F32R))

            xT0_sb = work.tile([128, DT, N], QKDT, tag="xT",
                               bufs=2 if qk_bf16 else 1, name="xT0_sb")
            load_xT_part(xT0_sb, 0, 0, 256)
            load_wqkv_part(2 * D, 384)        # V weights, first half
            load_xT_part(xT0_sb, 0, 256, 256)
            load_wqkv_part(2 * D + 384, 384)  # V weights, second half
            load_xT_part(xT0_sb, 0, 512, 256)
            load_wqkv_part(0, D)              # Q weights
            load_xT_part(xT0_sb, 0, 768, 256)
            load_wqkv_part(D, D)              # K weights
            wprojT_sb = constp.tile([128, DT, D], BF16)
            nc.sync.dma_start(wprojT_sb[:], wprojT_ext.rearrange("(o p) e -> p o e", p=128))
            bias_sb = constp.tile([128, D], F32)
            nc.sync.dma_start(bias_sb[:], bias_ext[:])

            def load_xT(b):
                xT_sb = work.tile([128, DT, N], QKDT, tag="xT", bufs=2 if qk_bf16 else 1, name="xT_sb")
                load_xT_part(xT_sb, b, 0, 512)
                load_xT_part(xT_sb, b, 512, 512)
                return xT_sb

            def alloc_v():
                return work.tile([128, 8, H, HD], BF16, tag="v", bufs=2 if qk_bf16 else 1, name="v_sb")

            def v_group(xT_sb, v_sb, rb, e0, ew):
                vps = mmps.tile([128, 512], F32, tag="mm", name="vps")
                for di in range(DT):
                    nc.tensor.matmul(
                        vps[:, :ew],
                        xT_sb[:, di, rb * 128:(rb + 1) * 128],
                        wqkvT_sb[:, di, 2 * D + e0:2 * D + e0 + ew],
                        start=(di == 0),
                        stop=(di == DT - 1),
                    )
                nc.vector.tensor_copy(
                    out=v_sb[:, rb, e0 // HD:(e0 + ew) // HD, :],
                    in_=vps[:, :ew].rearrange("p (h d) -> p h d", d=HD),
                )

            def emit_qk_group(xT_sb, qk_sb, t, e0, rc):
                qps = mmps.tile([128, 512], F32, tag="mm", name="qps")
                for di in range(DT):
                    nc.tensor.matmul(
                        qps[:],
                        wqkvT_sb[:, di, e0:e0 + 128],
                        xT_sb[:, di, rc * 512:(rc + 1) * 512],
                        start=(di == 0),
                        stop=(di == DT - 1),
                    )
                nc.vector.tensor_copy(
                    out=qk_sb[:, t, rc * 512:(rc + 1) * 512], in_=qps[:]
                )

            def alloc_qk():
                return work.tile([128, 2, N], QKDT, tag="qk", bufs=2, name="qk_sb")

            def qk_jobs(xT_sb, qk_sb, j):
                jobs = []
                for t, e0 in ((0, j * 128), (1, D + j * 128)):
                    for rc in range(2):
                        jobs.append((xT_sb, qk_sb, t, e0, rc))
                return jobs

            def emit_qk(xT_sb, j):
                qk_sb = alloc_qk()
                if probe == "qkoff":
                    nc.vector.memset(qk_sb[:], 1.0)
                    return qk_sb
                for args in qk_jobs(xT_sb, qk_sb, j):
                    emit_qk_group(*args)
                return qk_sb

            def emit_pv_kb(prev, kb):
                j, pT, pv, den_h, v_sb, _, _ = prev
                st = (kb == 0)
                sp = (kb == 7)
                nc.tensor.matmul(
                    pv[0:64, :], v_sb[:, kb, 2 * j, :], pT[:, kb, 0, :],
                    start=st, stop=sp,
                )
                nc.tensor.matmul(
                    pv[64:128, :], v_sb[:, kb, 2 * j + 1, :], pT[:, kb, 1, :],
                    start=st, stop=sp,
                )

            # softmax denominator: fold pT 8->1 over kb on DVE (bf16 tree),
            # then one Pool partition_all_reduce sums over partitions and
            # broadcasts - no PE work at all (the ones-matmul version cost
            # 16 accumulating F=512 PE matmuls per chunk).
            def emit_fold_a(prev):
                j, pT, _, _, _, _, fold = prev
                at = work.tile([128, 4, 2, 512], BF16, tag="foldA", bufs=1, name="at")
                nc.vector.tensor_tensor(at[:], pT[:, 0:4, :, :], pT[:, 4:8, :, :], ADD)
                fold["at"] = at

            def emit_fold_b(prev):
                fold = prev[6]
                at = fold["at"]
                bt = work.tile([128, 2, 2, 512], BF16, tag="foldB", bufs=1, name="bt")
                nc.vector.tensor_tensor(bt[:], at[:, 0:2, :, :], at[:, 2:4, :, :], ADD)
                fold["bt"] = bt

            def emit_fold_c(prev):
                fold = prev[6]
                bt = fold["bt"]
                ct = work.tile([128, 2, 512], BF16, tag="foldC", bufs=1, name="ct")
                nc.vector.tensor_tensor(ct[:], bt[:, 0, :, :], bt[:, 1, :, :], ADD)
                fold["ct"] = ct

            def emit_den(prev):
                # all-reduce ct over partitions on the (otherwise idle) Pool
                # engine; result is broadcast to every partition for free.
                fold = prev[6]
                ct = fold["ct"]
                ar = work.tile([128, 2, 512], F32, tag="allred", bufs=2, name="ar")
                nc.gpsimd.partition_all_reduce(
                    ar[:], ct[:], channels=128, reduce_op=bass_isa.ReduceOp.add)
                fold["ar"] = ar

            def emit_recip(prev):
                fold = prev[6]
                ar = fold["ar"]
                bcr = work.tile([128, 512], F32, tag="bcr", bufs=2, name="bcr")
                nc.vector.reciprocal(bcr[0:64, :], ar[0:64, 0, :])
                nc.vector.reciprocal(bcr[64:128, :], ar[64:128, 1, :])
                fold["bcr"] = bcr

            def emit_finalize(prev, ci):
                j, pT, pv, _, _, oT, fold = prev
                qc = ci % 2
                qsl = slice(qc * 512, (qc + 1) * 512)
                nc.vector.tensor_tensor(oT[:, j, qsl], pv[:], fold["bcr"][:], MUL)

            # ---- batch-pipelined emission ----
            xT_cur = xT0_sb
            v_cur = alloc_v()
            first_vjobs = []
            for rb in range(8):
                for e0, ew in ((0, 384), (384, 384)):
                    if rb < 6:
                        v_group(xT_cur, v_cur, rb, e0, ew)
                    else:
                        first_vjobs.append((xT_cur, v_cur, rb, e0, ew))

            pending_proj = []
            prev = None
            prev_ci = None
            for rep_b in range(NB):
                b = rep_b % BL
                projq = list(pending_proj)
                pending_proj = []
                # deferred work (interleaved into this batch's chunk slots)
                vjobs = list(first_vjobs)
                first_vjobs = []
                if qk_bf16 and rep_b + 1 < NB:
                    xT_next = load_xT((rep_b + 1) % BL)
                    v_next = alloc_v()
                    for rb in range(8):
                        for e0, ew in ((0, 384), (384, 384)):
                            vjobs.append((xT_next, v_next, rb, e0, ew))
                else:
                    xT_next = v_next = None

                oT_sb = work.tile([128, NP, N], BF16, tag="oT", bufs=2 if qk_bf16 else 1, name="oT_sb")

                # ---- output projection (bf16) + bias: deferred jobs ----
                def make_proj_job(oT_cur, b_cur, rb, split_dma=False):
                    def job():
                        out_sb = work.tile([128, D], F32, tag="outsb", bufs=3, name="out_sb")
                        if probe == "projoff":
                            nc.vector.tensor_tensor(out_sb[:], bias_sb[:], bias_sb[:], ADD)
                        else:
                            for e0, ew in ((0, 384), (384, 384)):
                                ops = mmps.tile([128, 512], F32, tag="mm", name="ops")
                                for di in range(DT):
                                    nc.tensor.matmul(
                                        ops[:, :ew],
                                        oT_cur[:, di, rb * 128:(rb + 1) * 128],
                                        wprojT_sb[:, di, e0:e0 + ew],
                                        start=(di == 0),
                                        stop=(di == DT - 1),
                                    )
                                nc.vector.tensor_tensor(
                                    out_sb[:, e0:e0 + ew], ops[:, :ew], bias_sb[:, e0:e0 + ew], ADD
                                )
                                if split_dma:
                                    nc.sync.dma_start(
                                        out_ext[b_cur * N + rb * 128:b_cur * N + (rb + 1) * 128, e0:e0 + ew],
                                        out_sb[:, e0:e0 + ew],
                                    )
                        if not split_dma:
                            nc.sync.dma_start(
                                out_ext[b_cur * N + rb * 128:b_cur * N + (rb + 1) * 128, :],
                                out_sb[:],
                            )
                    return job

                chunks = [(j, qc) for j in range(NP) for qc in range(2)]
                qk_tiles = {0: emit_qk(xT_cur, 0)}
                qkq = []
                for ci, (j, qc) in enumerate(chunks):
                    if qc == 0 and j + 1 < NP:
                        qk_tiles[j + 1] = emit_qk(xT_cur, j + 1)
                    qk_sb = qk_tiles[j]
                    qsl = slice(qc * 512, (qc + 1) * 512)
                    pT = work.tile([128, 8, 2, 512], BF16, tag="pT", bufs=2, name="pT")
                    # carry the pipeline across batches in the bf16 path; the
                    # f32r path has bufs=1 x/v tiles, so drain every batch
                    final = (j, qc) == chunks[-1] and (
                        rep_b + 1 == NB or not qk_bf16)
                    if final:
                        pv_f = pvps.tile([128, 512], F32, tag="pv", name="pv_f")
                        cur_f = (j, pT, pv_f, [], v_cur, oT_sb, {})
                    for kb in range(8):
                        ksl = slice(kb * 128, (kb + 1) * 128)
                        stp = stps.tile([128, 1024], F32, tag="stp", bufs=2, name="stp")
                        nc.tensor.matmul(
                            stp[:, 0:512], qk_sb[0:64, 1, ksl], qk_sb[0:64, 0, qsl],
                            start=True, stop=True,
                        )
                        nc.tensor.matmul(
                            stp[:, 512:1024], qk_sb[64:128, 1, ksl], qk_sb[64:128, 0, qsl],
                            start=True, stop=True,
                        )
                        if prev is not None:
                            emit_pv_kb(prev, kb)
                            if kb == 1:
                                emit_fold_a(prev)
                            elif kb == 3:
                                emit_fold_b(prev)
                            elif kb == 4:
                                emit_fold_c(prev)
                            elif kb == 5:
                                emit_den(prev)
                            elif kb == 6:
                                emit_recip(prev)
                        if final and kb >= 2:
                            emit_pv_kb(cur_f, kb - 2)
                        if kb in (3, 6) and vjobs:
                            xv, vv, rb, e0, ew = vjobs.pop(0)
                            v_group(xv, vv, rb, e0, ew)
                        if kb in (1, 5) and projq:
                            projq.pop(0)()
                        if kb in (0, 2, 4, 6) and qkq:
                            emit_qk_group(*qkq.pop(0))
                        nc.scalar.activation(
                            pT[:, kb, :, :],
                            stp[:].rearrange("p (h q) -> p h q", h=2),
                            EXP, scale=SCALE,
                        )
                    if prev is not None:
                        emit_finalize(prev, prev_ci)
                    if final:
                        # final drain: kb=7 PV, then the first half of the
                        # output projection (needs only qc=0 columns, all
                        # finalized) fills the PE while DVE/Pool run the fold
                        # chain for the last chunk.
                        emit_pv_kb(cur_f, 6)
                        emit_pv_kb(cur_f, 7)
                        fjobs = [
                            make_proj_job(oT_sb, b, rb, split_dma=(rb == 7))
                            for rb in range(8)
                        ]
                        emit_fold_a(cur_f)
                        emit_fold_b(cur_f)
                        emit_fold_c(cur_f)
                        emit_den(cur_f)
                        emit_recip(cur_f)
                        for rb in range(4):
                            fjobs[rb]()
                        emit_finalize(cur_f, ci)
                        for rb in range(4, 8):
                            fjobs[rb]()
                        prev = None
                    else:
                        pv = pvps.tile([128, 512], F32, tag="pv", name="pv")
                        prev = (j, pT, pv, [], v_cur, oT_sb, {})
                        prev_ci = ci

                # remaining deferred V groups
                for xv, vv, rb, e0, ew in vjobs:
                    v_group(xv, vv, rb, e0, ew)

                if qk_bf16 and rep_b + 1 < NB:
                    projjobs = [make_proj_job(oT_sb, b, rb) for rb in range(8)]
                else:
                    projjobs = []  # emitted in the final-chunk branch above
                pending_proj = projjobs

                for job in projq:
                    job()
                if (not qk_bf16) and rep_b + 1 < NB:
                    xT_next = load_xT((rep_b + 1) % BL)
                    v_next = alloc_v()
                    for rb in range(8):
                        for e0, ew in ((0, 384), (384, 384)):
                            v_group(xT_next, v_next, rb, e0, ew)
                if xT_next is not None:
                    xT_cur, v_cur = xT_next, v_next

    nc.compile()
    return nc


_CACHE = {}


def _get_nc():
    if "nc" not in _CACHE:
        _CACHE["nc"] = build_nc()
    return _CACHE["nc"]


def _prep_in_maps(x, w_qkv, w_proj, b_proj, qk_bf16=True):
    x = np.asarray(x, dtype=np.float32)
    w_qkv = np.asarray(w_qkv, dtype=np.float32)
    w_proj = np.asarray(w_proj, dtype=np.float32)
    b_proj = np.asarray(b_proj, dtype=np.float32)

    wqkvT = np.ascontiguousarray(w_qkv.T)                       # [768, 2304]
    if qk_bf16:
        wqkvT = wqkvT.astype(ml_dtypes.bfloat16)
    wprojT = np.ascontiguousarray(w_proj.T).astype(ml_dtypes.bfloat16)
    biasb = np.ascontiguousarray(np.broadcast_to(b_proj, (128, D)))

    in_maps = []
    for c in range(NCORES):
        xc = x[BL * c:BL * (c + 1)].reshape(ROWS, D)
        xTc = np.ascontiguousarray(xc.T)
        if qk_bf16:
            xTc = xTc.astype(ml_dtypes.bfloat16)
        in_maps.append({
            "xT": xTc,
            "wqkvT": wqkvT,
            "wprojT": wprojT,
            "biasb": biasb,
        })
    return in_maps


def kernel(x, w_qkv, w_proj, b_proj):
    nc = _get_nc()
    in_maps = _prep_in_maps(x, w_qkv, w_proj, b_proj)
    try:
        res = run_bass_kernel_spmd(nc, in_maps, core_ids=list(range(NCORES)))
    except Exception:
        # one retry for transient device/tunnel hiccups
        res = run_bass_kernel_spmd(nc, in_maps, core_ids=list(range(NCORES)))
    out = np.concatenate(
        [res.results[c]["out"].reshape(BL, N, D) for c in range(NCORES)], axis=0
    )
    return out



# revision 47
# speedup vs baseline: 1.0011x; 1.0011x over previous
"""Multi-head attention (B=16, N=1024, D=768, H=12) on 8 TRN2 NeuronCores.

Strategy: data-parallel over batch (2 batches per core, no collectives).
Per-core kernel, all matmuls on TensorE in bf16 (PE cost on TRN2 is
out-free-size cycles regardless of K/M, so every GEMM is tiled to its
F-cycle minimum):
  - QKV projection from pre-transposed x (feature-major xT in SBUF, bf16).
    Startup DMAs are split/interleaved (x quarters, V/Q/K weight sections)
    so the first V-projection matmul starts ~3us in.
  - Scores computed directly TRANSPOSED (S^T[k, q]) so the exp output
    P^T lands in exactly the layout the PV matmul needs as rhs.
  - exp on ScalarE with the 1/sqrt(hd) scale folded in (no max-subtract:
    scores are O(5) for this input distribution, far from fp32 overflow).
  - Softmax denominators WITHOUT PE work: P^T is folded 8->1 over the
    k-blocks by a 3-level bf16 add-tree on VectorE (runs at the 2x DVE
    rate), then a single partition_all_reduce on the idle GpSimd/Pool
    engine sums over partitions and broadcasts the result everywhere;
    the 1/denominator normalization is fused into the PV PSUM->SBUF
    copyback on VectorE.  (The previous ones-matmul denominator cost
    2x F=512 PE matmuls per k-block - 82us/core of pure PE overhead.)
  - PV col-tiled (two heads per PSUM bank, M=64 each) in bf16 producing
    O^T feature-major, which feeds the output projection (bf16) without
    any transposes.
  - Software pipelining: chunk c's PV/fold/finalize run interleaved into
    chunk c+1's score emission, carried ACROSS batch boundaries; the next
    batch's x DMA + V projection and the previous batch's output
    projection are interleaved into per-k-block slots, so the PE never
    drains between batches (95.9% PE occupancy, CoreSim).
"""

import sys

sys.path.insert(0, "/opt/trn_rl_repo")

import numpy as np
import ml_dtypes

import concourse.mybir as mybir
import concourse.tile as tile
from concourse import bacc
from concourse import bass_isa
from concourse.bass_utils import run_bass_kernel_spmd

F32 = mybir.dt.float32
F32R = mybir.dt.float32r
BF16 = mybir.dt.bfloat16

B, N, D = 16, 1024, 768
H = 12
HD = D // H          # 64
SCALE = float(HD) ** -0.5   # 0.125
NCORES = 8
BL = B // NCORES     # batches per core
ROWS = BL * N        # 2048 rows per core
DT = D // 128        # 6 d-tiles
NP = H // 2          # 6 head pairs
EXP = mybir.ActivationFunctionType.Exp
MUL = mybir.AluOpType.mult
ADD = mybir.AluOpType.add


def build_nc(repeat=1, qk_bf16=True, probe=None):
    nc = bacc.Bacc("TRN2", target_bir_lowering=False, debug=False)

    QKDT = BF16 if qk_bf16 else F32R
    xT_ext = nc.declare_dram_parameter("xT", [D, ROWS], BF16 if qk_bf16 else F32, isOutput=False)
    wqkvT_ext = nc.declare_dram_parameter("wqkvT", [D, 3 * D], BF16 if qk_bf16 else F32, isOutput=False)
    wprojT_ext = nc.declare_dram_parameter("wprojT", [D, D], BF16, isOutput=False)
    bias_ext = nc.declare_dram_parameter("biasb", [128, D], F32, isOutput=False)
    out_ext = nc.declare_dram_parameter("out", [ROWS, D], F32, isOutput=True)

    NB = repeat * BL

    with tile.TileContext(nc) as tc:
        with (
            tc.tile_pool(name="const", bufs=1) as constp,
            tc.tile_pool(name="work", bufs=1) as work,
            tc.tile_pool(name="mmps", bufs=2, space="PSUM") as mmps,
            tc.tile_pool(name="stps", bufs=2, space="PSUM") as stps,
            tc.tile_pool(name="pvps", bufs=2, space="PSUM") as pvps,
        ):
            # ---- constants / startup DMAs ----
            # Ordered so the PE can start as early as possible: first 512 rows
            # of x, then the V-section of wqkv (the first compute emitted is
            # the V projection), then the rest.
            def load_xT_part(xT_sb, b, h0, hl):
                xs = xT_ext[:, b * N + h0:b * N + h0 + hl].rearrange(
                    "(o p) r -> p o r", p=128)
                nc.sync.dma_start(
                    xT_sb[:, :, h0:h0 + hl], xs if qk_bf16 else xs.bitcast(F32R))

            wqkvT_sb = constp.tile([128, DT, 3 * D], QKDT)
            wq_src = wqkvT_ext.rearrange("(o p) e -> p o e", p=128)

            def load_wqkv_part(e0, ew):
                src = wq_src[:, :, e0:e0 + ew]
                nc.sync.dma_start(
                    wqkvT_sb[:, :, e0:e0 + ew], src if qk_bf16 else src.bitcast(F32R))

            xT0_sb = work.tile([128, DT, N], QKDT, tag="xT",
                               bufs=2 if qk_bf16 else 1, name="xT0_sb")
            load_xT_part(xT0_sb, 0, 0, 256)
            load_wqkv_part(2 * D, 384)        # V weights, first half
            load_xT_part(xT0_sb, 0, 256, 256)
            load_wqkv_part(2 * D + 384, 384)  # V weights, second half
            load_xT_part(xT0_sb, 0, 512, 256)
            load_wqkv_part(0, D)              # Q weights
            load_xT_part(xT0_sb, 0, 768, 256)
            load_wqkv_part(D, D)              # K weights
            wprojT_sb = constp.tile([128, DT, D], BF16)
            nc.sync.dma_start(wprojT_sb[:], wprojT_ext.rearrange("(o p) e -> p o e", p=128))
            bias_sb = constp.tile([128, D], F32)
            nc.sync.dma_start(bias_sb[:], bias_ext[:])

            def load_xT(b):
                xT_sb = work.tile([128, DT, N], QKDT, tag="xT", bufs=2 if qk_bf16 else 1, name="xT_sb")
                load_xT_part(xT_sb, b, 0, 512)
                load_xT_part(xT_sb, b, 512, 512)
                return xT_sb

            def alloc_v():
                return work.tile([128, 8, H, HD], BF16, tag="v", bufs=2 if qk_bf16 else 1, name="v_sb")

            def v_group(xT_sb, v_sb, rb, e0, ew):
                vps = mmps.tile([128, 512], F32, tag="mm", name="vps")
                for di in range(DT):
                    nc.tensor.matmul(
                        vps[:, :ew],
                        xT_sb[:, di, rb * 128:(rb + 1) * 128],
                        wqkvT_sb[:, di, 2 * D + e0:2 * D + e0 + ew],
                        start=(di == 0),
                        stop=(di == DT - 1),
                    )
                nc.vector.tensor_copy(
                    out=v_sb[:, rb, e0 // HD:(e0 + ew) // HD, :],
                    in_=vps[:, :ew].rearrange("p (h d) -> p h d", d=HD),
                )

            def emit_qk_group(xT_sb, qk_sb, t, e0, rc):
                qps = mmps.tile([128, 512], F32, tag="mm", name="qps")
                for di in range(DT):
                    nc.tensor.matmul(
                        qps[:],
                        wqkvT_sb[:, di, e0:e0 + 128],
                        xT_sb[:, di, rc * 512:(rc + 1) * 512],
                        start=(di == 0),
                        stop=(di == DT - 1),
                    )
                nc.vector.tensor_copy(
                    out=qk_sb[:, t, rc * 512:(rc + 1) * 512], in_=qps[:]
                )

            def alloc_qk():
                return work.tile([128, 2, N], QKDT, tag="qk", bufs=2, name="qk_sb")

            def qk_jobs(xT_sb, qk_sb, j):
                jobs = []
                for t, e0 in ((0, j * 128), (1, D + j * 128)):
                    for rc in range(2):
                        jobs.append((xT_sb, qk_sb, t, e0, rc))
                return jobs

            def emit_qk(xT_sb, j):
                qk_sb = alloc_qk()
                if probe == "qkoff":
                    nc.vector.memset(qk_sb[:], 1.0)
                    return qk_sb
                for args in qk_jobs(xT_sb, qk_sb, j):
                    emit_qk_group(*args)
                return qk_sb

            def emit_pv_kb(prev, kb):
                j, pT, pv, den_h, v_sb, _, _ = prev
                st = (kb == 0)
                sp = (kb == 7)
                nc.tensor.matmul(
                    pv[0:64, :], v_sb[:, kb, 2 * j, :], pT[:, kb, 0, :],
                    start=st, stop=sp,
                )
                nc.tensor.matmul(
                    pv[64:128, :], v_sb[:, kb, 2 * j + 1, :], pT[:, kb, 1, :],
                    start=st, stop=sp,
                )

            # softmax denominator: fold pT 8->1 over kb on DVE (bf16 tree),
            # then one Pool partition_all_reduce sums over partitions and
            # broadcasts - no PE work at all (the ones-matmul version cost
            # 16 accumulating F=512 PE matmuls per chunk).
            def emit_fold_a(prev):
                j, pT, _, _, _, _, fold = prev
                at = work.tile([128, 4, 2, 512], BF16, tag="foldA", bufs=1, name="at")
                nc.vector.tensor_tensor(at[:], pT[:, 0:4, :, :], pT[:, 4:8, :, :], ADD)
                fold["at"] = at

            def emit_fold_b(prev):
                fold = prev[6]
                at = fold["at"]
                bt = work.tile([128, 2, 2, 512], BF16, tag="foldB", bufs=1, name="bt")
                nc.vector.tensor_tensor(bt[:], at[:, 0:2, :, :], at[:, 2:4, :, :], ADD)
                fold["bt"] = bt

            def emit_fold_c(prev):
                fold = prev[6]
                bt = fold["bt"]
                ct = work.tile([128, 2, 512], BF16, tag="foldC", bufs=1, name="ct")
                nc.vector.tensor_tensor(ct[:], bt[:, 0, :, :], bt[:, 1, :, :], ADD)
                fold["ct"] = ct

            def emit_den(prev):
                # all-reduce ct over partitions on the (otherwise idle) Pool
                # engine; result is broadcast to every partition for free.
                fold = prev[6]
                ct = fold["ct"]
                ar = work.tile([128, 2, 512], F32, tag="allred", bufs=2, name="ar")
                nc.gpsimd.partition_all_reduce(
                    ar[:], ct[:], channels=128, reduce_op=bass_isa.ReduceOp.add)
                fold["ar"] = ar

            def emit_recip(prev):
                fold = prev[6]
                ar = fold["ar"]
                bcr = work.tile([128, 512], F32, tag="bcr", bufs=2, name="bcr")
                nc.vector.reciprocal(bcr[0:64, :], ar[0:64, 0, :])
                nc.vector.reciprocal(bcr[64:128, :], ar[64:128, 1, :])
                fold["bcr"] = bcr

            def emit_finalize(prev, ci):
                j, pT, pv, _, _, oT, fold = prev
                qc = ci % 2
                qsl = slice(qc * 512, (qc + 1) * 512)
                nc.vector.tensor_tensor(oT[:, j, qsl], pv[:], fold["bcr"][:], MUL)

            # ---- batch-pipelined emission ----
            xT_cur = xT0_sb
            v_cur = alloc_v()
            first_vjobs = []
            for rb in range(8):
                for e0, ew in ((0, 384), (384, 384)):
                    if rb < 6:
                        v_group(xT_cur, v_cur, rb, e0, ew)
                    else:
                        first_vjobs.append((xT_cur, v_cur, rb, e0, ew))

            pending_proj = []
            prev = None
            prev_ci = None
            for rep_b in range(NB):
                b = rep_b % BL
                projq = list(pending_proj)
                pending_proj = []
                # deferred work (interleaved into this batch's chunk slots)
                vjobs = list(first_vjobs)
                first_vjobs = []
                if qk_bf16 and rep_b + 1 < NB:
                    xT_next = load_xT((rep_b + 1) % BL)
                    v_next = alloc_v()
                    for rb in range(8):
                        for e0, ew in ((0, 384), (384, 384)):
                            vjobs.append((xT_next, v_next, rb, e0, ew))
                else:
                    xT_next = v_next = None

                oT_sb = work.tile([128, NP, N], BF16, tag="oT", bufs=2 if qk_bf16 else 1, name="oT_sb")

                # ---- output projection (bf16) + bias: deferred jobs ----
                def make_proj_job(oT_cur, b_cur, rb, split_dma=False):
                    def job():
                        out_sb = work.tile([128, D], F32, tag="outsb", bufs=3, name="out_sb")
                        if probe == "projoff":
                            nc.vector.tensor_tensor(out_sb[:], bias_sb[:], bias_sb[:], ADD)
                        else:
                            for e0, ew in ((0, 384), (384, 384)):
                                ops = mmps.tile([128, 512], F32, tag="mm", name="ops")
                                for di in range(DT):
                                    nc.tensor.matmul(
                                        ops[:, :ew],
                                        oT_cur[:, di, rb * 128:(rb + 1) * 128],
                                        wprojT_sb[:, di, e0:e0 + ew],
                                        start=(di == 0),
                                        stop=(di == DT - 1),
                                    )
                                nc.vector.tensor_tensor(
                                    out_sb[:, e0:e0 + ew], ops[:, :ew], bias_sb[:, e0:e0 + ew], ADD
                                )
                                if split_dma:
                                    nc.sync.dma_start(
                                        out_ext[b_cur * N + rb * 128:b_cur * N + (rb + 1) * 128, e0:e0 + ew],
                                        out_sb[:, e0:e0 + ew],
                                    )
                        if not split_dma:
                            nc.sync.dma_start(
                                out_ext[b_cur * N + rb * 128:b_cur * N + (rb + 1) * 128, :],
                                out_sb[:],
                            )
                    return job

                chunks = [(j, qc) for j in range(NP) for qc in range(2)]
                qk_tiles = {0: emit_qk(xT_cur, 0)}
                qkq = []
                for ci, (j, qc) in enumerate(chunks):
                    if qc == 0 and j + 1 < NP:
                        qk_tiles[j + 1] = emit_qk(xT_cur, j + 1)
                    qk_sb = qk_tiles[j]
                    qsl = slice(qc * 512, (qc + 1) * 512)
                    pT = work.tile([128, 8, 2, 512], BF16, tag="pT", bufs=2, name="pT")
                    # carry the pipeline across batches in the bf16 path; the
                    # f32r path has bufs=1 x/v tiles, so drain every batch
                    final = (j, qc) == chunks[-1] and (
                        rep_b + 1 == NB or not qk_bf16)
                    if final:
                        pv_f = pvps.tile([128, 512], F32, tag="pv", name="pv_f")
                        cur_f = (j, pT, pv_f, [], v_cur, oT_sb, {})
                    for kb in range(8):
                        ksl = slice(kb * 128, (kb + 1) * 128)
                        stp = stps.tile([128, 1024], F32, tag="stp", bufs=2, name="stp")
                        nc.tensor.matmul(
                            stp[:, 0:512], qk_sb[0:64, 1, ksl], qk_sb[0:64, 0, qsl],
                            start=True, stop=True,
                        )
                        nc.tensor.matmul(
                            stp[:, 512:1024], qk_sb[64:128, 1, ksl], qk_sb[64:128, 0, qsl],
                            start=True, stop=True,
                        )
                        if prev is not None:
                            emit_pv_kb(prev, kb)
                            if kb == 1:
                                emit_fold_a(prev)
                            elif kb == 3:
                                emit_fold_b(prev)
                            elif kb == 4:
                                emit_fold_c(prev)
                            elif kb == 5:
                                emit_den(prev)
                            elif kb == 6:
                                emit_recip(prev)
                        if final and kb >= 2:
                            emit_pv_kb(cur_f, kb - 2)
                        if final and kb == 3:
                            # incremental pairwise fold of the final chunk's
                            # pT as its exps land, so only a short chain
                            # remains after the kb loop
                            at_f = work.tile([128, 4, 2, 512], BF16,
                                             tag="foldA", bufs=1, name="at_f")
                            cur_f[6]["at"] = at_f
                            nc.vector.tensor_tensor(
                                at_f[:, 0], pT[:, 0], pT[:, 1], ADD)
                        elif final and kb == 5:
                            nc.vector.tensor_tensor(
                                cur_f[6]["at"][:, 1], pT[:, 2], pT[:, 3], ADD)
                        elif final and kb == 6:
                            at_f = cur_f[6]["at"]
                            nc.vector.tensor_tensor(
                                at_f[:, 2], pT[:, 4], pT[:, 5], ADD)
                            bt_f = work.tile([128, 2, 2, 512], BF16,
                                             tag="foldB", bufs=1, name="bt_f")
                            cur_f[6]["bt"] = bt_f
                            nc.vector.tensor_tensor(
                                bt_f[:, 0], at_f[:, 0], at_f[:, 1], ADD)
                        if kb in (3, 6) and vjobs:
                            xv, vv, rb, e0, ew = vjobs.pop(0)
                            v_group(xv, vv, rb, e0, ew)
                        if kb in (1, 5) and projq:
                            projq.pop(0)()
                        if kb in (0, 2, 4, 6) and qkq:
                            emit_qk_group(*qkq.pop(0))
                        nc.scalar.activation(
                            pT[:, kb, :, :],
                            stp[:].rearrange("p (h q) -> p h q", h=2),
                            EXP, scale=SCALE,
                        )
                    if prev is not None:
                        emit_finalize(prev, prev_ci)
                    if final:
                        # final drain: kb=7 PV, then the first half of the
                        # output projection (needs only qc=0 columns, all
                        # finalized) fills the PE while DVE/Pool run the fold
                        # chain for the last chunk.
                        fjobs = [
                            make_proj_job(oT_sb, b, rb, split_dma=(rb == 7))
                            for rb in range(8)
                        ]
                        at_f = cur_f[6]["at"]
                        bt_f = cur_f[6]["bt"]
                        nc.vector.tensor_tensor(
                            at_f[:, 3], pT[:, 6], pT[:, 7], ADD)
                        nc.vector.tensor_tensor(
                            bt_f[:, 1], at_f[:, 2], at_f[:, 3], ADD)
                        emit_fold_c(cur_f)
                        emit_den(cur_f)
                        emit_recip(cur_f)
                        fjobs[0]()
                        emit_pv_kb(cur_f, 6)
                        emit_pv_kb(cur_f, 7)
                        emit_finalize(cur_f, ci)
                        for rb in range(1, 8):
                            fjobs[rb]()
                        prev = None
                    else:
                        pv = pvps.tile([128, 512], F32, tag="pv", name="pv")
                        prev = (j, pT, pv, [], v_cur, oT_sb, {})
                        prev_ci = ci

                # remaining deferred V groups
                for xv, vv, rb, e0, ew in vjobs:
                    v_group(xv, vv, rb, e0, ew)

                if qk_bf16 and rep_b + 1 < NB:
                    projjobs = [make_proj_job(oT_sb, b, rb) for rb in range(8)]
                else:
                    projjobs = []  # emitted in the final-chunk branch above
                pending_proj = projjobs

                for job in projq:
                    job()
                if (not qk_bf16) and rep_b + 1 < NB:
                    xT_next = load_xT((rep_b + 1) % BL)
                    v_next = alloc_v()
                    for rb in range(8):
                        for e0, ew in ((0, 384), (384, 384)):
                            v_group(xT_next, v_next, rb, e0, ew)
                if xT_next is not None:
                    xT_cur, v_cur = xT_next, v_next

    nc.compile()
    return nc


_CACHE = {}


def _get_nc():
    if "nc" not in _CACHE:
        _CACHE["nc"] = build_nc()
    return _CACHE["nc"]


def _prep_in_maps(x, w_qkv, w_proj, b_proj, qk_bf16=True):
    x = np.asarray(x, dtype=np.float32)
    w_qkv = np.asarray(w_qkv, dtype=np.float32)
    w_proj = np.asarray(w_proj, dtype=np.float32)
    b_proj = np.asarray(b_proj, dtype=np.float32)

    wqkvT = np.ascontiguousarray(w_qkv.T)                       # [768, 2304]
    if qk_bf16:
        wqkvT = wqkvT.astype(ml_dtypes.bfloat16)
    wprojT = np.ascontiguousarray(w_proj.T).astype(ml_dtypes.bfloat16)
    biasb = np.ascontiguousarray(np.broadcast_to(b_proj, (128, D)))

    in_maps = []
    for c in range(NCORES):
        xc = x[BL * c:BL * (c + 1)].reshape(ROWS, D)
        xTc = np.ascontiguousarray(xc.T)
        if qk_bf16:
            xTc = xTc.astype(ml_dtypes.bfloat16)
        in_maps.append({
            "xT": xTc,
            "wqkvT": wqkvT,
            "wprojT": wprojT,
            "biasb": biasb,
        })
    return in_maps


def kernel(x, w_qkv, w_proj, b_proj):
    nc = _get_nc()
    in_maps = _prep_in_maps(x, w_qkv, w_proj, b_proj)
    try:
        res = run_bass_kernel_spmd(nc, in_maps, core_ids=list(range(NCORES)))
    except Exception:
        # one retry for transient device/tunnel hiccups
        res = run_bass_kernel_spmd(nc, in_maps, core_ids=list(range(NCORES)))
    out = np.concatenate(
        [res.results[c]["out"].reshape(BL, N, D) for c in range(NCORES)], axis=0
    )
    return out

